# revision 10
# baseline (speedup 1.0000x reference)
"""Binarized-MLP (BNN) kernel for Trainium2, data-parallel over batch on 8 NeuronCores.

Reference computation:
    h      = x @ sign(W1) + b1          x:[8192,4096] W1:[4096,512]
    logits = sign(h) @ sign(W2) + b2    W2:[512,10]
    out    = softmax(logits)            [8192,10]

Device strategy (per core, batch shard of 1024 rows):
  - x is supplied pre-transposed and split hi/lo in bf16 (x = hi + lo to
    ~2^-18 relative accuracy), so the dominant matmul runs as two bf16
    TensorE passes accumulating into the same fp32 PSUM bank — fp32-grade
    accuracy at bf16 speed.
  - Layout: stationary = sign(W1) f-tile [128f x 128j], moving = xT f-tile
    [128f x 512b] -> PSUM [128j x 512b]; all 8 PSUM banks hold the full
    per-core h [512 x 1024] and accumulate across the 32 f-tiles.
  - sign(h)+b1 is fused into one ScalarE Sign-activation (bias=b1) straight
    out of PSUM into bf16 SBUF tiles, already laid out [j, b] as the
    stationary operand of the second matmul.
  - Second matmul: stationary = sign(h) [128j x 128b], moving = sign(W2)
    [128j x 10] accumulated over 4 j-tiles -> PSUM [128b x 10].
  - Softmax on [128b, 10] tiles: add b2 (host-replicated [128,10]),
    reduce_max(negate) -> Exp activation with per-row bias and fused row-sum
    (accum_out), reciprocal, scale, DMA out.
"""

import numpy as np
import ml_dtypes

import concourse.bass as bass
import concourse.tile as tile
from concourse import mybir
from concourse.bass_utils import run_bass_kernel_spmd
from bass_rust import ScopedClock, VectorClock

BF16 = mybir.dt.bfloat16
F32 = mybir.dt.float32

B, F, H, C = 8192, 4096, 512, 10
NCORES = 8
BC = B // NCORES          # 1024 batch rows per core
NF = F // 128             # 32 f-tiles (contraction)
NJ = H // 128             # 4 j-tiles (hidden)
NBC = BC // 512           # 2 moving-operand chunks of 512
NBT = BC // 128           # 8 output b-tiles


class _PatchedTileContext(tile.TileContext):
    """Workaround for the walrus build in this container only accepting one
    sem wait on a CTRL-type (Drain) instruction: spread the exit drain's
    per-proc waits across several drains with one wait each."""

    def _drain_and_barrier(self, tick_clock, wait_clock):
        gc = tick_clock.global_clock
        ticks = list(gc)
        nprocs = len(ticks)
        for i, t in enumerate(ticks):
            if t == 0:
                continue
            partial = [0] * nprocs
            partial[i] = t
            inst = self.nc.sync.drain()
            wait_clock.add_sem_waits(
                inst.ins, ScopedClock({None: VectorClock(partial)})
            )
        self.nc.sync.drain()

        self.nc.all_engine_barrier()
        assert self.sems is not None
        popped = self.nc._tile_sem_poison_stack.pop()
        assert popped is self._sem_poison
        self.nc.clear_and_free_semaphores(list(self.sems.allocated().values()))
        self.nc.all_engine_barrier()


def _split_waits_json(raw: bytes) -> bytes:
    """The walrus build in this container accepts at most ONE sem wait per
    instruction (bass's own wait_op asserts the same). Tile attaches several.
    Rewrite the serialized BIR: excess waits become standalone EventSemaphore
    wait instructions on the same engine immediately before the instruction —
    semantically identical, since the engine blocks there first."""
    import json as _json

    m = _json.loads(raw)
    ctr = 0
    for fn in m.get("functions", []):
        for bb in fn.get("blocks", []):
            insts = bb.get("instructions", [])
            new_insts = []
            for inst in insts:
                si = inst.get("sync_info")
                waits = si.get("on_wait") or [] if si else []
                if len(waits) > 1:
                    for w in waits[:-1]:
                        new_insts.append(
                            {
                                "debug": inst.get("debug", 0),
                                "engine": inst["engine"],
                                "ins": [],
                                "outs": [],
                                "name": f"WSPLIT-{ctr}",
                                "opcode": "EventSemaphore",
                                "sync_info": {"on_update": [], "on_wait": [w]},
                            }
                        )
                        ctr += 1
                    si["on_wait"] = [waits[-1]]
                new_insts.append(inst)
            bb["instructions"] = new_insts
    return _json.dumps(m).encode()


def _install_wait_splitter(nc: bass.Bass) -> None:
    orig = nc.to_json_bytes

    def patched():
        return _split_waits_json(orig())

    nc.to_json_bytes = patched


def build_kernel() -> bass.Bass:
    nc = bass.Bass()
    # xt packs hi|lo along the free dim: [:, 0:1024] = bf16(x.T), [:, 1024:2048] = residual
    xt = nc.dram_tensor("xt", [F, 2 * BC], BF16, kind="ExternalInput")
    w1 = nc.dram_tensor("w1", [F, H], BF16, kind="ExternalInput")
    w2 = nc.dram_tensor("w2", [H, C], F32, kind="ExternalInput")
    b1 = nc.dram_tensor("b1", [H, 1], F32, kind="ExternalInput")
    b2r = nc.dram_tensor("b2r", [128, C], F32, kind="ExternalInput")
    out = nc.dram_tensor("out", [BC, C], F32, kind="ExternalOutput")

    with _PatchedTileContext(nc) as tc:
        with (
            tc.tile_pool(name="consts", bufs=1) as consts,
            tc.tile_pool(name="w1raw", bufs=3) as w1raw_pool,
            tc.tile_pool(name="w1s", bufs=NF) as w1s_pool,
            tc.tile_pool(name="xin", bufs=3) as xin_pool,
            tc.tile_pool(name="signh", bufs=NJ * NBC) as signh_pool,
            tc.tile_pool(name="psum", bufs=8, space="PSUM") as psum_pool,
            tc.tile_pool(name="smx", bufs=4) as smx_pool,
            tc.tile_pool(name="outp", bufs=4) as out_pool,
        ):
            # ---- constants ----
            b2_t = consts.tile([128, C], F32, name="b2", tag="b2")
            nc.sync.dma_start(b2_t[:], b2r[:, :])
            b1_t = []
            for j in range(NJ):
                t = consts.tile([128, 1], F32, name="b1", tag="b1")
                nc.sync.dma_start(t[:], b1[j * 128:(j + 1) * 128, :])
                b1_t.append(t)
            w2s_t = []
            for j in range(NJ):
                raw = consts.tile([128, C], F32, name="w2raw", tag="w2raw")
                nc.sync.dma_start(raw[:], w2[j * 128:(j + 1) * 128, :])
                s = consts.tile([128, C], BF16, name="w2s", tag="w2s")
                nc.scalar.sign(s[:], raw[:])
                w2s_t.append(s)

            # ---- stage B: h = x @ sign(W1), all 8 PSUM banks accumulate ----
            psumB = [
                [psum_pool.tile([128, 512], F32, name="psB", tag="psB") for _ in range(NBC)]
                for _ in range(NJ)
            ]
            w1s_tiles = []
            for f in range(NF):
                raw = w1raw_pool.tile([128, H], BF16, name="w1raw", tag="w1raw")
                nc.sync.dma_start(raw[:], w1[f * 128:(f + 1) * 128, :])
                w1s = w1s_pool.tile([128, H], BF16, name="w1s", tag="w1s")
                nc.scalar.sign(w1s[:], raw[:])
                w1s_tiles.append(w1s)

                xf = xin_pool.tile([128, 2 * BC], BF16, name="xin", tag="xin")
                nc.sync.dma_start(xf[:], xt[f * 128:(f + 1) * 128, :])

                first = f == 0
                last = f == NF - 1
                for j in range(NJ):
                    lhs = w1s[:, j * 128:(j + 1) * 128]
                    for bc in range(NBC):
                        hi = xf[:, bc * 512:(bc + 1) * 512]
                        lo = xf[:, BC + bc * 512:BC + (bc + 1) * 512]
                        nc.tensor.matmul(
                            psumB[j][bc][:], lhs, hi, start=first, stop=False
                        )
                        nc.tensor.matmul(
                            psumB[j][bc][:], lhs, lo, start=False, stop=last
                        )

            # ---- stage C: sign(h + b1) -> bf16 [j, b] tiles ----
            signh = [[None] * NBC for _ in range(NJ)]
            for j in range(NJ):
                for bc in range(NBC):
                    s = signh_pool.tile([128, 512], BF16, name="signh", tag="signh")
                    nc.scalar.sign(s[:], psumB[j][bc][:], bias=b1_t[j][:])
                    signh[j][bc] = s

            # ---- stage D+E: logits, softmax, out ----
            for bt in range(NBT):
                bc, col = bt // 4, (bt % 4) * 128
                ps2 = psum_pool.tile([128, C], F32, name="psD", tag="psB")
                for j in range(NJ):
                    nc.tensor.matmul(
                        ps2[:],
                        signh[j][bc][:, col:col + 128],
                        w2s_t[j][:],
                        start=(j == 0),
                        stop=(j == NJ - 1),
                    )
                logits = smx_pool.tile([128, C], F32, name="logits", tag="logits")
                nc.vector.tensor_add(logits[:], ps2[:], b2_t[:])
                negmax = smx_pool.tile([128, 1], F32, name="negmax", tag="negmax")
                nc.vector.reduce_max(
                    negmax[:], logits[:], axis=mybir.AxisListType.X, negate=True
                )
                e = smx_pool.tile([128, C], F32, name="e", tag="e")
                ssum = smx_pool.tile([128, 1], F32, name="ssum", tag="ssum")
                nc.scalar.activation(
                    e[:],
                    logits[:],
                    mybir.ActivationFunctionType.Exp,
                    bias=negmax[:],
                    accum_out=ssum[:],
                )
                rs = smx_pool.tile([128, 1], F32, name="rs", tag="rs")
                nc.vector.reciprocal(rs[:], ssum[:])
                o = out_pool.tile([128, C], F32, name="o", tag="o")
                nc.vector.tensor_scalar_mul(o[:], e[:], rs[:])
                nc.sync.dma_start(out[bt * 128:(bt + 1) * 128, :], o[:])

    _install_wait_splitter(nc)
    return nc


_cached_nc = None


def _get_nc() -> bass.Bass:
    global _cached_nc
    if _cached_nc is None:
        _cached_nc = build_kernel()
    return _cached_nc


def kernel(inputs, W1, b1, W2, b2):
    x = np.ascontiguousarray(np.asarray(inputs, dtype=np.float32))
    W1 = np.asarray(W1, dtype=np.float32)
    b1 = np.asarray(b1, dtype=np.float32)
    W2 = np.ascontiguousarray(np.asarray(W2, dtype=np.float32))
    b2 = np.asarray(b2, dtype=np.float32)

    w1_bf = np.ascontiguousarray(W1.astype(ml_dtypes.bfloat16))
    b1_col = np.ascontiguousarray(b1.reshape(H, 1))
    b2_rep = np.ascontiguousarray(np.broadcast_to(b2.reshape(1, C), (128, C)))

    in_maps = []
    for c in range(NCORES):
        xc_t = x[c * BC:(c + 1) * BC, :].T  # [F, BC]
        hi = xc_t.astype(ml_dtypes.bfloat16)
        lo = (xc_t - hi.astype(np.float32)).astype(ml_dtypes.bfloat16)
        xt = np.empty((F, 2 * BC), dtype=ml_dtypes.bfloat16)
        xt[:, :BC] = hi
        xt[:, BC:] = lo
        in_maps.append(
            {"xt": xt, "w1": w1_bf, "w2": W2, "b1": b1_col, "b2r": b2_rep}
        )

    nc = _get_nc()
    res = run_bass_kernel_spmd(nc, in_maps, core_ids=list(range(NCORES)))
    global last_results
    last_results = res
    out = np.concatenate([res.results[c]["out"] for c in range(NCORES)], axis=0)
    return out.astype(np.float32)


last_results = None


# revision 12
# speedup vs baseline: 1.0038x; 1.0038x over previous
"""Binarized-MLP (BNN) kernel for Trainium2, data-parallel over batch on 8 NeuronCores.

Reference computation:
    h      = x @ sign(W1) + b1          x:[8192,4096] W1:[4096,512]
    logits = sign(h) @ sign(W2) + b2    W2:[512,10]
    out    = softmax(logits)            [8192,10]

Device strategy (per core, batch shard of 1024 rows):
  - x is supplied pre-transposed and split hi/lo in bf16 (x = hi + lo to
    ~2^-18 relative accuracy), so the dominant matmul runs as two bf16
    TensorE passes accumulating into the same fp32 PSUM bank — fp32-grade
    accuracy at bf16 speed.
  - Layout: stationary = sign(W1) f-tile [128f x 128j], moving = xT f-tile
    [128f x 512b] -> PSUM [128j x 512b]; all 8 PSUM banks hold the full
    per-core h [512 x 1024] and accumulate across the 32 f-tiles.
  - sign(h)+b1 is fused into one ScalarE Sign-activation (bias=b1) straight
    out of PSUM into bf16 SBUF tiles, already laid out [j, b] as the
    stationary operand of the second matmul.
  - Second matmul: stationary = sign(h) [128j x 128b], moving = sign(W2)
    [128j x 10] accumulated over 4 j-tiles -> PSUM [128b x 10].
  - Softmax on [128b, 10] tiles: add b2 (host-replicated [128,10]),
    reduce_max(negate) -> Exp activation with per-row bias and fused row-sum
    (accum_out), reciprocal, scale, DMA out.
"""

import numpy as np
import ml_dtypes

import concourse.bass as bass
import concourse.tile as tile
from concourse import mybir
from concourse.bass_utils import run_bass_kernel_spmd
from bass_rust import ScopedClock, VectorClock

BF16 = mybir.dt.bfloat16
F32 = mybir.dt.float32

B, F, H, C = 8192, 4096, 512, 10
NCORES = 8
BC = B // NCORES          # 1024 batch rows per core
NF = F // 128             # 32 f-tiles (contraction)
NJ = H // 128             # 4 j-tiles (hidden)
NBC = BC // 512           # 2 moving-operand chunks of 512
NBT = BC // 128           # 8 output b-tiles


class _PatchedTileContext(tile.TileContext):
    """Workaround for the walrus build in this container only accepting one
    sem wait on a CTRL-type (Drain) instruction: spread the exit drain's
    per-proc waits across several drains with one wait each."""

    def _drain_and_barrier(self, tick_clock, wait_clock):
        gc = tick_clock.global_clock
        ticks = list(gc)
        nprocs = len(ticks)
        for i, t in enumerate(ticks):
            if t == 0:
                continue
            partial = [0] * nprocs
            partial[i] = t
            inst = self.nc.sync.drain()
            wait_clock.add_sem_waits(
                inst.ins, ScopedClock({None: VectorClock(partial)})
            )
        self.nc.sync.drain()

        self.nc.all_engine_barrier()
        assert self.sems is not None
        popped = self.nc._tile_sem_poison_stack.pop()
        assert popped is self._sem_poison
        self.nc.clear_and_free_semaphores(list(self.sems.allocated().values()))
        self.nc.all_engine_barrier()


def _split_waits_json(raw: bytes) -> bytes:
    """The walrus build in this container accepts at most ONE sem wait per
    instruction (bass's own wait_op asserts the same). Tile attaches several.
    Rewrite the serialized BIR: excess waits become standalone EventSemaphore
    wait instructions on the same engine immediately before the instruction —
    semantically identical, since the engine blocks there first."""
    import json as _json

    m = _json.loads(raw)
    ctr = 0
    for fn in m.get("functions", []):
        for bb in fn.get("blocks", []):
            insts = bb.get("instructions", [])
            new_insts = []
            for inst in insts:
                si = inst.get("sync_info")
                waits = si.get("on_wait") or [] if si else []
                if len(waits) > 1:
                    for w in waits[:-1]:
                        new_insts.append(
                            {
                                "debug": inst.get("debug", 0),
                                "engine": inst["engine"],
                                "ins": [],
                                "outs": [],
                                "name": f"WSPLIT-{ctr}",
                                "opcode": "EventSemaphore",
                                "sync_info": {"on_update": [], "on_wait": [w]},
                            }
                        )
                        ctr += 1
                    si["on_wait"] = [waits[-1]]
                new_insts.append(inst)
            bb["instructions"] = new_insts
    return _json.dumps(m).encode()


def _install_wait_splitter(nc: bass.Bass) -> None:
    orig = nc.to_json_bytes

    def patched():
        return _split_waits_json(orig())

    nc.to_json_bytes = patched


def build_kernel() -> bass.Bass:
    nc = bass.Bass()
    # xt packs hi|lo along the free dim: [:, 0:1024] = bf16(x.T), [:, 1024:2048] = residual
    xt = nc.dram_tensor("xt", [F, 2 * BC], BF16, kind="ExternalInput")
    w1 = nc.dram_tensor("w1", [F, H], BF16, kind="ExternalInput")
    w2 = nc.dram_tensor("w2", [H, C], F32, kind="ExternalInput")
    b1 = nc.dram_tensor("b1", [H, 1], F32, kind="ExternalInput")
    b2r = nc.dram_tensor("b2r", [128, C], F32, kind="ExternalInput")
    out = nc.dram_tensor("out", [BC, C], F32, kind="ExternalOutput")

    with _PatchedTileContext(nc) as tc:
        with (
            tc.tile_pool(name="consts", bufs=1) as consts,
            tc.tile_pool(name="w1raw", bufs=3) as w1raw_pool,
            tc.tile_pool(name="w1s", bufs=NF) as w1s_pool,
            tc.tile_pool(name="xin", bufs=3) as xin_pool,
            tc.tile_pool(name="signh", bufs=NJ * NBC) as signh_pool,
            tc.tile_pool(name="psum", bufs=8, space="PSUM") as psum_pool,
            tc.tile_pool(name="smx", bufs=4) as smx_pool,
            tc.tile_pool(name="outp", bufs=4) as out_pool,
        ):
            # ---- constants ----
            tiny = consts.tile([128, 1], F32, name="tiny", tag="tiny")
            nc.vector.memset(tiny[:], 1e-30)
            b2_t = consts.tile([128, C], F32, name="b2", tag="b2")
            nc.sync.dma_start(b2_t[:], b2r[:, :])
            b1_t = []
            for j in range(NJ):
                t = consts.tile([128, 1], F32, name="b1", tag="b1")
                nc.sync.dma_start(t[:], b1[j * 128:(j + 1) * 128, :])
                b1_t.append(t)
            w2s_t = []
            for j in range(NJ):
                raw = consts.tile([128, C], F32, name="w2raw", tag="w2raw")
                nc.sync.dma_start(raw[:], w2[j * 128:(j + 1) * 128, :])
                s = consts.tile([128, C], BF16, name="w2s", tag="w2s")
                nc.scalar.sign(s[:], raw[:], bias=tiny[:])
                w2s_t.append(s)

            # ---- stage B: h = x @ sign(W1), all 8 PSUM banks accumulate ----
            psumB = [
                [psum_pool.tile([128, 512], F32, name="psB", tag="psB") for _ in range(NBC)]
                for _ in range(NJ)
            ]
            w1s_tiles = []
            for f in range(NF):
                raw = w1raw_pool.tile([128, H], BF16, name="w1raw", tag="w1raw")
                nc.sync.dma_start(raw[:], w1[f * 128:(f + 1) * 128, :])
                w1s = w1s_pool.tile([128, H], BF16, name="w1s", tag="w1s")
                nc.scalar.sign(w1s[:], raw[:], bias=tiny[:])
                w1s_tiles.append(w1s)

                xf = xin_pool.tile([128, 2 * BC], BF16, name="xin", tag="xin")
                nc.sync.dma_start(xf[:], xt[f * 128:(f + 1) * 128, :])

                first = f == 0
                last = f == NF - 1
                for j in range(NJ):
                    lhs = w1s[:, j * 128:(j + 1) * 128]
                    for bc in range(NBC):
                        hi = xf[:, bc * 512:(bc + 1) * 512]
                        lo = xf[:, BC + bc * 512:BC + (bc + 1) * 512]
                        nc.tensor.matmul(
                            psumB[j][bc][:], lhs, hi, start=first, stop=False
                        )
                        nc.tensor.matmul(
                            psumB[j][bc][:], lhs, lo, start=False, stop=last
                        )

            # ---- stage C: sign(h + b1) -> bf16 [j, b] tiles ----
            signh = [[None] * NBC for _ in range(NJ)]
            for j in range(NJ):
                for bc in range(NBC):
                    s = signh_pool.tile([128, 512], BF16, name="signh", tag="signh")
                    nc.scalar.sign(s[:], psumB[j][bc][:], bias=b1_t[j][:])
                    signh[j][bc] = s

            # ---- stage D+E: logits, softmax, out ----
            for bt in range(NBT):
                bc, col = bt // 4, (bt % 4) * 128
                ps2 = psum_pool.tile([128, C], F32, name="psD", tag="psB")
                for j in range(NJ):
                    nc.tensor.matmul(
                        ps2[:],
                        signh[j][bc][:, col:col + 128],
                        w2s_t[j][:],
                        start=(j == 0),
                        stop=(j == NJ - 1),
                    )
                logits = smx_pool.tile([128, C], F32, name="logits", tag="logits")
                nc.vector.tensor_add(logits[:], ps2[:], b2_t[:])
                negmax = smx_pool.tile([128, 1], F32, name="negmax", tag="negmax")
                nc.vector.reduce_max(
                    negmax[:], logits[:], axis=mybir.AxisListType.X, negate=True
                )
                e = smx_pool.tile([128, C], F32, name="e", tag="e")
                ssum = smx_pool.tile([128, 1], F32, name="ssum", tag="ssum")
                nc.scalar.activation(
                    e[:],
                    logits[:],
                    mybir.ActivationFunctionType.Exp,
                    bias=negmax[:],
                    accum_out=ssum[:],
                )
                rs = smx_pool.tile([128, 1], F32, name="rs", tag="rs")
                nc.vector.reciprocal(rs[:], ssum[:])
                o = out_pool.tile([128, C], F32, name="o", tag="o")
                nc.vector.tensor_scalar_mul(o[:], e[:], rs[:])
                nc.sync.dma_start(out[bt * 128:(bt + 1) * 128, :], o[:])

    _install_wait_splitter(nc)
    return nc


_cached_nc = None


def _get_nc() -> bass.Bass:
    global _cached_nc
    if _cached_nc is None:
        _cached_nc = build_kernel()
    return _cached_nc


def kernel(inputs, W1, b1, W2, b2):
    x = np.ascontiguousarray(np.asarray(inputs, dtype=np.float32))
    W1 = np.asarray(W1, dtype=np.float32)
    b1 = np.asarray(b1, dtype=np.float32)
    W2 = np.ascontiguousarray(np.asarray(W2, dtype=np.float32))
    b2 = np.asarray(b2, dtype=np.float32)

    w1_bf = np.ascontiguousarray(W1.astype(ml_dtypes.bfloat16))
    b1_col = np.ascontiguousarray(b1.reshape(H, 1))
    b2_rep = np.ascontiguousarray(np.broadcast_to(b2.reshape(1, C), (128, C)))

    in_maps = []
    for c in range(NCORES):
        xc_t = x[c * BC:(c + 1) * BC, :].T  # [F, BC]
        hi = xc_t.astype(ml_dtypes.bfloat16)
        lo = (xc_t - hi.astype(np.float32)).astype(ml_dtypes.bfloat16)
        xt = np.empty((F, 2 * BC), dtype=ml_dtypes.bfloat16)
        xt[:, :BC] = hi
        xt[:, BC:] = lo
        in_maps.append(
            {"xt": xt, "w1": w1_bf, "w2": W2, "b1": b1_col, "b2r": b2_rep}
        )

    nc = _get_nc()
    res = run_bass_kernel_spmd(nc, in_maps, core_ids=list(range(NCORES)))
    global last_results
    last_results = res
    out = np.concatenate([res.results[c]["out"] for c in range(NCORES)], axis=0)
    return out.astype(np.float32)


last_results = None
